# revision 2
# baseline (speedup 1.0000x reference)
"""Trainium2 Bass kernel for nn_EntropyModel (MoE routing over K=4 class towers).

Strategy: every op in the tower is a per-pixel 1x1 conv (matmul over channels),
and the final one-hot masked sum selects exactly one class tower per pixel.
So route on the host: sort pixels by seg class, give each of the 8 cores half
of one class's pixels (expert-parallel, 2 cores per class), run that class's
tower densely on its gathered pixels, and scatter the results back.

The 5-matmul tower is algebraically collapsed to 4 matmuls per pixel by
folding the linear layers around the two LeakyReLUs (host precomputes the
merged 128x128 weights):
    a2 = lrelu(V x + c)          V  = Wr1 W1,      c   = Wr1 b1 + br1
    h3 = lrelu(T x + U a2 + b3') T  = W3 W1,       U   = W3 Wr2,
                                 b3' = W3 (b1 + br2) + b3
    y  = W4 h3 + b4
Matmuls run in float32r (reduced-precision fp32 PE mode, ~1e-4 rel err per
matmul, 4x faster than full fp32).
"""
import numpy as np

import concourse.mybir as mybir
import concourse.tile as tile
from concourse import bacc
from concourse.bass_utils import run_bass_kernel_spmd

B, C, H, W = 2, 128, 192, 192
K = 4
O = 60
NTOT = B * H * W
NCORES = 8
MACRO = 2048  # free-dim per macro tile (4 PSUM banks)
MMF = 512     # free-dim per matmul (1 PSUM bank, fp32)

F32 = mybir.dt.float32
F32R = mybir.dt.float32r

LAST_RESULTS = None  # test harness reads exec_time_ns off this

_nc_cache = {}


def _build(cap):
    nc = bacc.Bacc(None, target_bir_lowering=False)
    x = nc.dram_tensor("x", [C, cap], F32R, kind="ExternalInput")
    vt = nc.dram_tensor("vt", [C, C], F32R, kind="ExternalInput")
    tt = nc.dram_tensor("tt", [C, C], F32R, kind="ExternalInput")
    ut = nc.dram_tensor("ut", [C, C], F32R, kind="ExternalInput")
    w4t = nc.dram_tensor("w4t", [C, O], F32R, kind="ExternalInput")
    cb = nc.dram_tensor("cb", [C, 1], F32, kind="ExternalInput")
    b3 = nc.dram_tensor("b3", [C, 1], F32, kind="ExternalInput")
    b4 = nc.dram_tensor("b4", [O, 1], F32, kind="ExternalInput")
    y = nc.dram_tensor("y", [O, cap], F32, kind="ExternalOutput")

    spans = []
    s = 0
    while s < cap:
        w = min(MACRO, cap - s)
        spans.append((s, w))
        s += w

    Lrelu = mybir.ActivationFunctionType.Lrelu

    with tile.TileContext(nc) as tc:
        with tc.tile_pool(name="const", bufs=1) as cw, \
             tc.tile_pool(name="xin", bufs=3) as xp, \
             tc.tile_pool(name="mid", bufs=2) as mp, \
             tc.tile_pool(name="yout", bufs=2) as yp, \
             tc.tile_pool(name="ps", bufs=2, space="PSUM") as ps:
            vtt = cw.tile([C, C], F32R)
            nc.sync.dma_start(vtt[:], vt[:])
            ttt = cw.tile([C, C], F32R)
            nc.sync.dma_start(ttt[:], tt[:])
            utt = cw.tile([C, C], F32R)
            nc.sync.dma_start(utt[:], ut[:])
            w4tt = cw.tile([C, O], F32R)
            nc.sync.dma_start(w4tt[:], w4t[:])
            cbt = cw.tile([C, 1], F32)
            nc.sync.dma_start(cbt[:], cb[:])
            b3t = cw.tile([C, 1], F32)
            nc.sync.dma_start(b3t[:], b3[:])
            b4t = cw.tile([O, 1], F32)
            nc.sync.dma_start(b4t[:], b4[:])

            for s, w in spans:
                xt = xp.tile([C, MACRO], F32R, tag="x", name="xt")[:, :w]
                nc.sync.dma_start(xt[:], x[:, s:s + w])

                pa = ps.tile([C, MACRO], F32, tag="mm", name="pa")[:, :w]
                for j in range(0, w, MMF):
                    nc.tensor.matmul(pa[:, j:j + MMF], vtt[:], xt[:, j:j + MMF],
                                     start=True, stop=True)
                a2 = mp.tile([C, MACRO], F32R, tag="a2", name="a2")[:, :w]
                nc.scalar.activation(a2[:], pa[:], Lrelu,
                                     bias=cbt[:, 0:1], scale=1.0, alpha=0.01)

                ph = ps.tile([C, MACRO], F32, tag="mm", name="ph")[:, :w]
                for j in range(0, w, MMF):
                    nc.tensor.matmul(ph[:, j:j + MMF], ttt[:], xt[:, j:j + MMF],
                                     start=True, stop=False)
                for j in range(0, w, MMF):
                    nc.tensor.matmul(ph[:, j:j + MMF], utt[:], a2[:, j:j + MMF],
                                     start=False, stop=True)
                h3 = mp.tile([C, MACRO], F32R, tag="h3", name="h3")[:, :w]
                nc.scalar.activation(h3[:], ph[:], Lrelu,
                                     bias=b3t[:, 0:1], scale=1.0, alpha=0.01)

                py = ps.tile([O, MACRO], F32, tag="mm", name="py")[:, :w]
                for j in range(0, w, MMF):
                    nc.tensor.matmul(py[:, j:j + MMF], w4tt[:], h3[:, j:j + MMF],
                                     start=True, stop=True)
                yt = yp.tile([O, MACRO], F32, tag="y", name="yt")[:, :w]
                nc.vector.tensor_scalar_add(yt[:], py[:], b4t[:, 0:1])
                nc.sync.dma_start(y[:, s:s + w], yt[:])
    nc.compile()
    return nc


def kernel(fusion_context, seg, W1, b1, Wr1, br1, Wr2, br2, W3, b3, W4, b4):
    global LAST_RESULTS
    fusion_context = np.asarray(fusion_context, dtype=np.float32)
    seg = np.asarray(seg)

    # [B,C,H,W] -> [C, B*H*W]; column n = (b, h, w) row-major
    xcols = np.ascontiguousarray(
        fusion_context.transpose(1, 0, 2, 3).reshape(C, NTOT))
    segf = seg.reshape(-1).astype(np.int64)

    # Route: per class index list, split into two halves -> 8 core shards
    shards = []  # (class_id, column_indices)
    for k in range(K):
        ix = np.nonzero(segf == k)[0]
        h = (len(ix) + 1) // 2
        shards.append((k, ix[:h]))
        shards.append((k, ix[h:]))
    assert len(shards) == NCORES

    cap = max(len(ix) for _, ix in shards)
    cap = max(MMF, -(-cap // MMF) * MMF)  # round up to matmul tile

    if cap not in _nc_cache:
        _nc_cache[cap] = _build(cap)
    nc = _nc_cache[cap]

    f64 = np.float64
    in_maps = []
    for k, ix in shards:
        xs = np.zeros((C, cap), dtype=np.float32)
        xs[:, :len(ix)] = xcols[:, ix]
        V = W1[k].astype(f64).T @ Wr1[k].astype(f64).T    # (Wr1 W1)^T
        T = W1[k].astype(f64).T @ W3[k].astype(f64).T     # (W3 W1)^T
        U = Wr2[k].astype(f64).T @ W3[k].astype(f64).T    # (W3 Wr2)^T
        c = Wr1[k].astype(f64) @ b1[k].astype(f64) + br1[k].astype(f64)
        b3p = W3[k].astype(f64) @ (b1[k].astype(f64) + br2[k].astype(f64)) \
            + b3[k].astype(f64)
        in_maps.append({
            "x": xs,
            "vt": np.ascontiguousarray(V, dtype=np.float32),
            "tt": np.ascontiguousarray(T, dtype=np.float32),
            "ut": np.ascontiguousarray(U, dtype=np.float32),
            "w4t": np.ascontiguousarray(W4[k].T, dtype=np.float32),
            "cb": c.astype(np.float32).reshape(C, 1),
            "b3": b3p.astype(np.float32).reshape(C, 1),
            "b4": np.ascontiguousarray(b4[k], dtype=np.float32).reshape(O, 1),
        })

    res = run_bass_kernel_spmd(nc, in_maps, core_ids=list(range(NCORES)))
    LAST_RESULTS = res

    out = np.empty((O, NTOT), dtype=np.float32)
    for (k, ix), r in zip(shards, res.results):
        out[:, ix] = r["y"][:, :len(ix)]
    return np.ascontiguousarray(
        out.reshape(O, B, H * W).transpose(1, 0, 2).reshape(B, O, H, W))
